# revision 35
# baseline (speedup 1.0000x reference)
"""Trainium2 Bass kernel for causal self-attention (B=4, T=2048, C=1024, H=16).

Sharding: 2 heads per core across 8 cores (tensor parallel on heads).
Per core:
  1. QKV projection for its 128 channels (2 heads), q/k/v kept transposed
     [ch, tok] in SBUF (fp16 matmuls at full PE rate).
  2. Flash-style causal attention per (batch, head) in fp16. Scores are
     computed TRANSPOSED (S^T [s, t]) so the softmax denominator comes out
     of the same matmul that applies V: lhsT = [v_h | ones] makes PSUM rows
     64:128 the row-sum Z (no max subtraction: |S*scale| < ~6).
     The two heads' score matmuls are row-tiled (tile_position 0/64) so they
     run concurrently in the PE array; their exps are fused into one
     activation call over a 2-bank PSUM tile.
  3. V is transposed [tok, ch] via the DMA xbar (dma_start_transpose), not
     the PE, freeing the tensor engine.
  4. y^T blocks (fp16) are exchanged with an on-chip AllToAll so each core
     owns a 1024-token slice, then projected with the full Wp in fp16.
Emission is software-pipelined: the projection of batch b+1 and the output
projection of ready epochs are interleaved unit-by-unit into batch b's
attention so the PE never idles on the scalar engine's exp.
Host side: x is pre-transposed, per-core weight slices pre-sliced; output
slices are concatenated and bp added at the end.
"""

import itertools

import numpy as np

import concourse.bass as bass
import concourse.mybir as mybir
import concourse.tile as tile
from concourse import bacc

F32 = mybir.dt.float32
F16 = mybir.dt.float16
EXP = mybir.ActivationFunctionType.Exp

# problem shape (hardcoded per harness contract)
B, T, C, H = 4, 2048, 1024, 16
D = C // H              # 64
NCORES = 8
BT = B * T
TSL = BT // NCORES      # tokens per core after AllToAll
SCALE = 1.0 / np.sqrt(np.float32(D))

AV_DEPTH = 2            # score/exp blocks in flight ahead of their AV matmul


def build_program(b=B, t=T, c=C, ncores=NCORES, reps=1, no_collective=False):
    """Build the SPMD single-core program. Requires c == 128 * ncores."""
    assert c == 128 * ncores, "2 heads of 64 dims per core"
    bt = b * t
    tsl = bt // ncores
    nk = c // 128            # contraction tiles for projections
    tch = t // 512           # 512-token chunks per batch
    sbk = t // 128           # 128-token s-blocks per batch
    n_out_ch = tsl // 512    # local out-proj token chunks

    nc = bacc.Bacc("TRN2", target_bir_lowering=False, num_devices=ncores)

    xT = nc.dram_tensor("xT", [c, bt], F16, kind="ExternalInput")
    wqT = nc.dram_tensor("wqT", [c, 128], F16, kind="ExternalInput")
    wkT = nc.dram_tensor("wkT", [c, 128], F16, kind="ExternalInput")
    wvT = nc.dram_tensor("wvT", [c, 128], F16, kind="ExternalInput")
    bq = nc.dram_tensor("bq", [128, 1], F32, kind="ExternalInput")
    bk = nc.dram_tensor("bk", [128, 1], F32, kind="ExternalInput")
    bv = nc.dram_tensor("bv", [128, 1], F32, kind="ExternalInput")
    wpT = nc.dram_tensor("wpT", [c, c], F16, kind="ExternalInput")
    outT = nc.dram_tensor("outT", [c, tsl], F32, kind="ExternalOutput")

    with tile.TileContext(nc) as tc:
        with (
            tc.tile_pool(name="singles", bufs=1) as singles,
            tc.tile_pool(name="dram", bufs=1, space="DRAM") as dram,
            tc.tile_pool(name="xin", bufs=8) as xin,
            tc.tile_pool(name="qkv", bufs=2) as qkv,
            tc.tile_pool(name="vva", bufs=2) as vva,
            tc.tile_pool(name="ptile", bufs=4) as ptile,
            tc.tile_pool(name="ynorm", bufs=3) as ynorm,
            tc.tile_pool(name="outsb", bufs=3) as outsb,
            tc.tile_pool(name="yg", bufs=3) as ygpool,
            tc.tile_pool(name="ps_s", bufs=2, space="PSUM") as ps_s,
            tc.tile_pool(name="ps_y", bufs=1, space="PSUM") as ps_y,
            tc.tile_pool(name="ps_mm", bufs=2, space="PSUM") as ps_mm,
        ):
            # one AllToAll epoch per batch: each core ends up owning a
            # tsl/b = 256-token slice of that batch's tokens
            eptok = tsl // b
            a2a_ins, a2a_outs = [], []
            for e in range(b):
                a2a_in_e = dram.tile([ncores, 128, eptok], F16,
                                     name=f"a2a_in{e}")
                a2a_out_e = dram.tile([ncores, 128, eptok], F16,
                                      name=f"a2a_out{e}")
                a2a_ins.append(a2a_in_e)
                a2a_outs.append(a2a_out_e)

            # --- constants ---
            # causal mask for the two heads' diagonal 128x128 sub-blocks:
            # keep (0) where t - s >= 0 else -1e10
            cmask2 = singles.tile([128, 2, 128], F32)
            nc.gpsimd.memset(cmask2, 0.0)
            for h in range(2):
                nc.gpsimd.affine_select(
                    out=cmask2[:, h, :], in_=cmask2[:, h, :],
                    compare_op=mybir.AluOpType.is_ge,
                    fill=-1e10, base=0, channel_multiplier=-1,
                    pattern=[[1, 128]],
                )
            # input-side loads go on the SP DMA queue; transposes, staging
            # and stores go on the Activation DMA queue so slow dependent
            # DMAs never head-of-line-block the x loads.
            # only wq/bq load before the first x chunk; k/v weights follow it
            # so the first projection matmul starts as early as possible
            w_tiles = {}
            bias_tiles = {}
            w_srcs = {}
            for nm, wt, bias in (("q", wqT, bq), ("k", wkT, bk), ("v", wvT, bv)):
                w_tiles[nm] = singles.tile([128, nk, 128], F16, name=f"w{nm}")
                bias_tiles[nm] = singles.tile([128, 1], F32, name=f"b{nm}")
                w_srcs[nm] = (wt, bias)

            def emit_w_load(nm):
                wt, bias = w_srcs[nm]
                nc.sync.dma_start(
                    out=w_tiles[nm],
                    in_=wt.rearrange("(ck p) o -> p ck o", p=128))
                nc.sync.dma_start(out=bias_tiles[nm], in_=bias[:, :])

            # wq in two halves so the very first matmul starts sooner
            wt_q, bias_q = w_srcs["q"]
            wq_view = wt_q.rearrange("(ck p) o -> p ck o", p=128)
            h0 = nk // 2
            nc.sync.dma_start(out=w_tiles["q"][:, 0:h0, :],
                              in_=wq_view[:, 0:h0, :])
            nc.sync.dma_start(out=bias_tiles["q"], in_=bias_q[:, :])
            nc.sync.dma_start(out=w_tiles["q"][:, h0:nk, :],
                              in_=wq_view[:, h0:nk, :])
            wp_all = singles.tile([128, nk, c], F16)
            wp_loaded = [False]

            def emit_wp_load():
                if not wp_loaded[0]:
                    wp_loaded[0] = True
                    # scheduler hint: keep this 2MB load off the DMA engines
                    # during the latency-critical cold start
                    with tc.tile_wait_until(0.015):
                        nc.scalar.dma_start(
                            out=wp_all,
                            in_=wpT.rearrange("(ck p) o -> p ck o", p=128))
            kv_loaded = [False]

            xview = xT.rearrange("(ck p) g -> p ck g", p=128)

            def alloc_bufs():
                qT = qkv.tile([128, t], F16, tag="qT")
                kT = qkv.tile([128, t], F16, tag="kT")
                vT = qkv.tile([128, t], F16, tag="vT")
                # per s-block layout: [v_h0 (64) | ones (64) | v_h1 (64)]:
                # both heads' AV lhsT are then contiguous slices sharing the
                # ones block -- h0 = [0:128] ([v|1]), h1 = [64:192] ([1|v],
                # so h1's Z lands in PSUM rows 0:64 and y in 64:128)
                vv = vva.tile([128, sbk, 192], F16, tag="vv")
                return qT, kT, vT, vv

            xts = {}

            def xt_gen(bi, split_first=False):
                """Pre-issue the x-chunk loads for batch bi (one batch
                ahead) so projection matmuls never wait on the DMA queue."""
                t0 = bi * t
                for j in range(tch):
                    xt = xin.tile([128, nk, 512], F16, tag="xt")
                    xts[bi, j] = xt
                    cols = slice(t0 + 512 * j, t0 + 512 * j + 512)
                    if split_first and j == 0:
                        h = nk // 2
                        nc.sync.dma_start(
                            out=xt[:, 0:h, :], in_=xview[:, 0:h, cols])
                        nc.sync.dma_start(
                            out=xt[:, h:nk, :], in_=xview[:, h:nk, cols])
                    else:
                        nc.sync.dma_start(out=xt, in_=xview[:, :, cols])
                    if not kv_loaded[0]:
                        kv_loaded[0] = True
                        emit_w_load("k")
                        emit_w_load("v")
                    yield

            def proj_gen(bi, bufs):
                """QKV projection for batch bi, unit per yield."""
                qT, kT, vT, vv = bufs
                nc.vector.memset(vv[:, :, 64:128], 1.0)
                emit_wp_load()
                yield
                for j in range(tch):
                    xt = xts.pop((bi, j))
                    for nm, dst in (("q", qT), ("k", kT), ("v", vT)):
                        ps = ps_mm.tile([128, 512], F32, tag="mm")
                        for ck in range(nk):
                            nc.tensor.matmul(
                                ps, w_tiles[nm][:, ck, :], xt[:, ck, :],
                                start=(ck == 0), stop=(ck == nk - 1))
                        nc.vector.tensor_scalar_add(
                            dst[:, 512 * j:512 * j + 512], ps, bias_tiles[nm])
                        yield

            def vvt_gen(bufs):
                """V transposes via the DMA xbar, emitted once the batch's
                projection is fully emitted so their dispatch waits never
                head-of-line-block the x loads; head h lands at
                vv[:, i, 128h:128h+64]."""
                qT, kT, vT, vv = bufs
                for j in range(tch):
                    for i in range(4 * j, 4 * j + 4):
                        nc.sync.dma_start_transpose(
                            out=vv[:, i, 0:64],
                            in_=vT[0:64, 128 * i:128 * i + 128])
                        nc.sync.dma_start_transpose(
                            out=vv[:, i, 128:192],
                            in_=vT[64:128, 128 * i:128 * i + 128])
                    yield

            # score/exp -> AV software pipeline, carried across chunk AND
            # batch boundaries so neither the PE nor the exp queue ever
            # drains at a seam
            pending = []

            def emit_scores(bufs, j, i):
                qT, kT, vT, vv = bufs
                toff = max(0, 128 * i - 512 * j)
                w = 512 - toff
                spx = ps_s.tile([128, 2, 512], F32, tag="spx")
                for h in range(2):
                    d0 = 64 * h
                    nc.tensor.matmul(
                        spx[:, h, :w],
                        kT[d0:d0 + 64, 128 * i:128 * i + 128],
                        qT[d0:d0 + 64, 512 * j + toff:512 * j + 512],
                        start=True, stop=True,
                        tile_position=(d0, 0),
                    )
                if 128 * i >= 512 * j:
                    nc.vector.tensor_add(
                        spx[:, :, 0:128], spx[:, :, 0:128], cmask2)
                pt = ptile.tile([128, 2, 512], F16, tag="pt")
                nc.scalar.activation(
                    pt[:, :, :w], spx[:, :, :w], EXP, scale=float(SCALE))
                return (pt, toff, w)

            def emit_av(ent):
                bi, vv, j, i, nsb, yps, (pt, toff, w) = ent
                for h in range(2):
                    lhsT = (vv[:, i, 0:128] if h == 0
                            else vv[:, i, 64:192])
                    nc.tensor.matmul(
                        yps[h][:, toff:512],
                        lhsT,
                        pt[:, h, :w],
                        start=(i == 0), stop=(i == nsb - 1),
                        skip_group_check=True,
                    )
                if i == nsb - 1:
                    finish_chunk(bi, j, yps)

            def finish_chunk(bi, j, yps):
                # normalize by Z (PSUM rows 64:128) and stage for the
                # per-batch AllToAll: chunk j's 512 tokens go to cores
                # 2j and 2j+1 as two 256-token halves. Staging DMAs ride
                # the idle GPSIMD SWDGE queue so they can't block the
                # x-load or exp queues.
                yt2 = ynorm.tile([128, 512], F16, tag="yt2")
                for h in range(2):
                    # h0 PSUM rows: [y | Z]; h1 rows: [Z | y] (ones-first)
                    zrow, yrow = (64, 0) if h == 0 else (0, 64)
                    zr = ynorm.tile([64, 512], F32, tag="zr")
                    nc.vector.reciprocal(zr, yps[h][zrow:zrow + 64, :])
                    nc.vector.tensor_mul(
                        yt2[64 * h:64 * h + 64, :],
                        yps[h][yrow:yrow + 64, :], zr)
                for half in range(2):
                    nc.gpsimd.dma_start(
                        out=a2a_ins[bi][2 * j + half, :, 0:eptok],
                        in_=yt2[:, eptok * half:eptok * half + eptok])
                if j == tch - 1:
                    # the batch's AllToAll must be emitted after the staging
                    # of its last chunk, which (with the cross-batch AV
                    # pipeline) happens a couple of blocks into the next
                    # batch's emission — so emit it here, not in the driver.
                    # The epoch's gather for the output projection follows
                    # immediately (same pool ring, correct order).
                    emit_collective(bi)
                    ygall = ygpool.tile([128, nk, eptok], F16, tag="ygall")
                    nc.gpsimd.dma_start(
                        out=ygall,
                        in_=a2a_outs[bi].rearrange("s p g -> p s g"))
                    ygalls[bi] = ygall

            def attn_gen(bi, bufs, flush=False):
                """Causal attention for batch bi, one (i,j) block per yield."""
                vv = bufs[3]
                for j in range(tch):
                    yp0 = ps_y.tile([128, 512], F32, tag="yp0")
                    yp1 = ps_y.tile([128, 512], F32, tag="yp1")
                    yps = [yp0, yp1]
                    nsb = 4 * j + 4
                    for i in range(nsb):
                        pending.append(
                            (bi, vv, j, i, nsb, yps,
                             emit_scores(bufs, j, i)))
                        if len(pending) > AV_DEPTH:
                            emit_av(pending.pop(0))
                        yield
                while pending and (flush or len(pending) > AV_DEPTH):
                    emit_av(pending.pop(0))
                    yield

            ygalls = {}

            def outproj_gen(ep):
                """Project the local 256-token slice of epoch ep with Wp."""
                ygall = ygalls.pop(ep)
                for ot in range(nk):
                    ops = ps_mm.tile([128, eptok], F32, tag="mm")
                    for ck in range(nk):
                        nc.tensor.matmul(
                            ops, wp_all[:, ck, 128 * ot:128 * ot + 128],
                            ygall[:, ck, :],
                            start=(ck == 0), stop=(ck == nk - 1))
                    osb = outsb.tile([128, eptok], F32, tag="osb")
                    nc.vector.tensor_copy(osb, ops)
                    nc.sync.dma_start(
                        out=outT[128 * ot:128 * ot + 128,
                                 eptok * ep:eptok * ep + eptok],
                        in_=osb)
                    yield

            def emit_collective(ep):
                if no_collective:
                    # local stand-in with the same DMA volume (for
                    # TimelineSim, which rejects collectives); on the pool
                    # queue like the real collective_compute
                    nc.gpsimd.dma_start(out=a2a_outs[ep], in_=a2a_ins[ep])
                else:
                    nc.gpsimd.collective_compute(
                        "AllToAll", mybir.AluOpType.bypass,
                        replica_groups=[list(range(ncores))],
                        ins=[a2a_ins[ep].opt()], outs=[a2a_outs[ep].opt()],
                    )

            def skip_gen(n):
                for _ in range(n):
                    yield

            for _rep in range(reps):
                cur = alloc_bufs()
                for _ in xt_gen(0, split_first=(_rep == 0)):
                    pass
                for _ in xt_gen(1):
                    pass
                for _ in proj_gen(0, cur):
                    pass
                vvt0 = vvt_gen(cur)
                next(vvt0)  # chunk-0 V blocks must land before attention
                for bi in range(b):
                    side_gens = []
                    if bi == 0:
                        side_gens.append(vvt0)
                    if bi + 1 < b:
                        nxt = alloc_bufs()
                        side_gens.append(proj_gen(bi + 1, nxt))
                    else:
                        nxt = None
                        side_gens.append(skip_gen(18))
                    if bi + 2 < b:
                        side_gens.append(xt_gen(bi + 2))
                    if bi + 1 < b:
                        side_gens.append(vvt_gen(nxt))
                    if bi >= 1:
                        side_gens.append(outproj_gen(bi - 1))
                    side = itertools.chain(*side_gens)
                    for _ in attn_gen(bi, cur, flush=(bi == b - 1)):
                        next(side, None)
                    for _ in side:
                        pass
                    cur = nxt
                for _ in outproj_gen(b - 1):
                    pass
    nc.compile()
    return nc


_PROGRAM_CACHE = {}


def _get_program(key=(B, T, C, NCORES)):
    if key not in _PROGRAM_CACHE:
        _PROGRAM_CACHE[key] = build_program(*key)
    return _PROGRAM_CACHE[key]


def make_in_maps(x, Wq, bq, Wk, bk, Wv, bv, Wp, ncores=NCORES):
    bt = x.shape[0] * x.shape[1]
    c = x.shape[2]
    xT = np.ascontiguousarray(x.reshape(bt, c).T.astype(np.float16))
    wpT = np.ascontiguousarray(Wp.T.astype(np.float16))
    in_maps = []
    for core in range(ncores):
        s = slice(128 * core, 128 * core + 128)
        in_maps.append({
            "xT": xT,
            "wqT": np.ascontiguousarray(Wq[s, :].T.astype(np.float16)),
            "wkT": np.ascontiguousarray(Wk[s, :].T.astype(np.float16)),
            "wvT": np.ascontiguousarray(Wv[s, :].T.astype(np.float16)),
            "bq": np.ascontiguousarray(bq[s].reshape(128, 1), dtype=np.float32),
            "bk": np.ascontiguousarray(bk[s].reshape(128, 1), dtype=np.float32),
            "bv": np.ascontiguousarray(bv[s].reshape(128, 1), dtype=np.float32),
            "wpT": wpT,
        })
    return in_maps


def assemble_output(results, b=B, t=T, c=C, bp=None):
    bt = b * t
    n = len(results)
    eptok = (bt // n) // b
    out = np.empty((bt, c), np.float32)
    for core, res in enumerate(results):
        oT = res["outT"]
        for e in range(b):
            r0 = e * t + eptok * core
            out[r0:r0 + eptok, :] = oT[:, eptok * e:eptok * e + eptok].T
    out = out.reshape(b, t, c)
    if bp is not None:
        out = out + bp
    return out


def kernel(x, Wk, bk, Wq, bq, Wv, bv, Wp, bp, _trace=False):
    from concourse.bass_utils import run_bass_kernel_spmd

    x = np.asarray(x, np.float32)
    nc = _get_program()
    in_maps = make_in_maps(x, np.asarray(Wq), np.asarray(bq), np.asarray(Wk),
                           np.asarray(bk), np.asarray(Wv), np.asarray(bv),
                           np.asarray(Wp))
    res = run_bass_kernel_spmd(nc, in_maps, list(range(NCORES)), trace=_trace)
    out = assemble_output(res.results, bp=np.asarray(bp, np.float32))
    if _trace:
        return out, res
    return out
